# revision 35
# baseline (speedup 1.0000x reference)
"""Contrastive loss (NT-Xent style) Trainium2 kernel, 8-core SPMD.

Math: with z_i = normalize(instance_emb.reshape(4096, 512)),
zbag = normalize(bag_emb) [8, 512], z_j = repeat(zbag, 512) and
Z = [z_i; z_j] (8192 rows), the reference computes

  loss = (1/8192) * sum_r [ log(sum_{c != r} exp(2*sim[r,c])) - 2*pos[r] ]

with sim = Z @ Z.T, pos[r] = sim[r, r +- 4096].  Only the
G = z_i @ z_i.T quadrant (4096x4096) needs dense compute (the z_j half
has 8 distinct rows); S1 = z_i @ zbag.T [4096, 8] and Bg = zbag @ zbag.T
[8, 8] cover the rest:

  denom_i[r] = sum_c exp(2 G[r,c]) - e^2 + 512 * sum_b exp(2 S1[r,b])
  denom_j[b] = sum_r exp(2 S1[r,b]) + 512 * sum_b' exp(2 Bg[b,b']) - e^2
  loss*8192 = sum_r [log denom_i[r] - 4*S1[r, r//512]]
            + 512*sum_b log denom_j[b]

Distribution: NO collectives (the runtime entry barrier + AllGather
dominated the collective version).  Every core gets the full input,
rotated so its own 512 rows are local rows 0:511, and computes
E = exp(2 G) only for COLUMN blocks at relative offsets d in {0..4}
(20 column-tiles of 128).  E is symmetric, so each computed off-diagonal
entry serves two denominators: the activation accumulator gives the
column-block partials (denom of the E-row index) and a ones-matmul over
partitions gives the row partials (denom of the core's own rows).
Offsets 1..3 cover their mirror offsets 7..5; offset 4 is computed by
both members of each pair and the host discards the copy from cores
4..7.  The host sums the per-core partials (the gather/unshard step).

Per core c the host sends np.roll(Y, -512c):
  - yt0/yt1: RAW y^T fp8e4, packed for DoubleRow, only the first 2560
             rotated columns: ytm[p, j, col] = y[col, (2m+j)*128 + p]
  - yrow:    y row-major bf16 tiled [128, 20, 512] (tile t = rotated
             rows 128t..128t+127) -- feeds sumsq (rinv) and own z.
Device, per column-tile t (20 tiles):
  u[c, r] = sum_d y[c, d] * z_own[r, d]   (raw lhsT; 2 fp8 DR matmuls)
  E = exp(u * (2*rinv_c))                 (column norm folded into the
                                           ACT per-partition scale)
  pd[c, t] = sum_r E                      (ACT accumulator)
  tiles 4..19 also: E -> SBUF bf16, pr += ones^T @ E  (row partials,
  separate PSUM accumulators for d in {1,2,3} and d = 4)
Host: denom rows = rolled pd partials + own-row pr partials + S1/Bg
terms, then log/sum in float64.
"""

import os
import numpy as np
import ml_dtypes
from contextlib import ExitStack

import concourse.bass as bass
import concourse.bacc as bacc
import concourse.tile as tile
from concourse import mybir
from concourse import bass_utils
from concourse.masks import make_identity

F32 = mybir.dt.float32
BF16 = mybir.dt.bfloat16
FP8 = mybir.dt.float8e4

NP_FP8 = ml_dtypes.float8_e4m3
NP_BF16 = ml_dtypes.bfloat16

B, N, D = 8, 512, 512
BS = B * N              # 4096 instance rows
NCORES = 8
RPC = BS // NCORES      # 512 own rows per core
CT = 20                 # computed column tiles (offsets 0..4)
CCOLS = CT * 128        # 2560 columns
E2 = float(np.exp(2.0))
AF = mybir.ActivationFunctionType
ALU = mybir.AluOpType
DR = mybir.MatmulPerfMode.DoubleRow
SEED = 2.0 / float(np.sqrt(512.0))

LAST_EXEC_TIME_NS = None
_CACHED_NC = None


def _build_kernel(nc):
    yt0 = nc.dram_tensor("yt0", [128, 2, CCOLS], FP8, kind="ExternalInput")
    yt1 = nc.dram_tensor("yt1", [128, 2, CCOLS], FP8, kind="ExternalInput")
    yrow = nc.dram_tensor("yrow", [128, CT, D], BF16, kind="ExternalInput")
    bag = nc.dram_tensor("bag", [B, D], F32, kind="ExternalInput")
    pd_d = nc.dram_tensor("pd", [128, CT], F32, kind="ExternalOutput")
    pr_d = nc.dram_tensor("pr", [1, 2 * RPC], F32, kind="ExternalOutput")
    s1_d = nc.dram_tensor("s1t", [B, RPC], F32, kind="ExternalOutput")
    bg_d = nc.dram_tensor("bg", [B, B], F32, kind="ExternalOutput")

    with tile.TileContext(nc) as tc:
        _body(tc, yt0.ap(), yt1.ap(), yrow.ap(), bag.ap(),
              pd_d.ap(), pr_d.ap(), s1_d.ap(), bg_d.ap())
    return nc


def _body(tc, yt0, yt1, yrow, bag, pd_d, pr_d, s1_d, bg_d):
    nc = tc.nc
    with ExitStack() as ctx:
        consts = ctx.enter_context(tc.tile_pool(name="consts", bufs=1))
        persist = ctx.enter_context(tc.tile_pool(name="persist", bufs=1))
        zpool = ctx.enter_context(tc.tile_pool(name="zpool", bufs=4))
        sqp = ctx.enter_context(tc.tile_pool(name="sqp", bufs=2))
        etp = ctx.enter_context(tc.tile_pool(name="etp", bufs=4))
        ps_g = ctx.enter_context(tc.tile_pool(name="ps_g", bufs=4, space="PSUM"))
        ps_tr = ctx.enter_context(tc.tile_pool(name="ps_tr", bufs=2, space="PSUM"))
        ps_sm = ctx.enter_context(tc.tile_pool(name="ps_sm", bufs=1, space="PSUM"))
        ps_pr = ctx.enter_context(tc.tile_pool(name="ps_pr", bufs=1, space="PSUM"))

        identw = consts.tile([128, 128], BF16, name="identw")
        make_identity(nc, identw)
        identb = consts.tile([B, B], BF16, name="identb")
        make_identity(nc, identb)
        ones = consts.tile([128, 1], BF16, name="ones")
        nc.gpsimd.memset(ones, 1.0)
        # dummy Ln: hoists the natural_log act-table load (Square/Copy are
        # in that set too) into the idle DMA-wait window, leaving a single
        # on-chain table swap (to the exp set) later
        lnw = consts.tile([1, 1], F32, name="lnw")
        nc.scalar.activation(lnw, ones[0:1, 0:1], AF.Ln)
        bln2 = consts.tile([128, 1], F32, name="bln2")
        nc.gpsimd.memset(bln2, -0.5 * float(np.log(2.0)))

        # ---- input DMAs: own 4-tile slab first (it gates the prologue) ----
        yrall = persist.tile([128, CT, D], BF16, name="yrall")
        ytm = [persist.tile([128, 2, CCOLS], FP8, name=f"ytm_{m}") for m in range(2)]
        bag_t = persist.tile([B, D], F32, name="bag_t")

        nc.sync.dma_start(out=yrall[:, 0:4, :], in_=yrow[:, 0:4, :])
        yts = [yt0, yt1]
        for cchunk in range(2):
            sl = slice(cchunk * 1280, (cchunk + 1) * 1280)
            for m in range(2):
                nc.sync.dma_start(out=ytm[m][:, :, sl], in_=yts[m][:, :, sl])
        nc.scalar.dma_start(out=yrall[:, 4:12, :], in_=yrow[:, 4:12, :])
        nc.gpsimd.dma_start(out=bag_t, in_=bag[:, :])
        nc.gpsimd.dma_start(out=yrall[:, 12:20, :], in_=yrow[:, 12:20, :])

        bno = persist.tile([128, CT, 6], F32, name="bno")
        rinv2 = persist.tile([128, CT], F32, name="rinv2")  # 2/||row||

        def bn_tiles(ts):
            for t in ts:
                nc.vector.bn_stats(bno[:, t, :], yrall[:, t, :])

        # rinv2 = 2*ss^-1/2, table-free: sumsq of 512 N(0,1) terms is
        # 512 +- ~16%, so 3 Newton steps from the constant seed 2/sqrt(512)
        # converge to ~1e-5 (no scalar Ln/Exp => no act-table swaps).
        # In r2-space (r2 = 2r): r2 <- r2*(1.5 - (ss/8)*r2^2).
        def newton(eng, r2, ssf, a, iters=2):
            eng.tensor_scalar(
                out=r2, in0=ssf, scalar1=-(SEED ** 3), scalar2=1.5 * SEED,
                op0=ALU.mult, op1=ALU.add,
            )
            for _ in range(iters):
                eng.tensor_mul(a, r2, r2)
                eng.tensor_mul(a, a, ssf)
                eng.tensor_scalar(
                    out=a, in0=a, scalar1=-1.0, scalar2=1.5,
                    op0=ALU.mult, op1=ALU.add,
                )
                eng.tensor_mul(r2, r2, a)

        def rsqrt_batch(sl, tag, eng=None):
            # bn post: ss/8 = 32*(mu_e^2+mu_o^2) + (m2e+m2o)/8
            eng = eng or nc.gpsimd
            w = sl.stop - sl.start
            t1 = sqp.tile([128, w], F32, name=f"t1_{tag}")
            t2 = sqp.tile([128, w], F32, name=f"t2_{tag}")
            s0 = sqp.tile([128, w], F32, name=f"s0_{tag}")
            ssf = sqp.tile([128, w], F32, name=f"ssf_{tag}")
            eng.tensor_mul(t1, bno[:, sl, 1], bno[:, sl, 1])
            eng.tensor_mul(t2, bno[:, sl, 4], bno[:, sl, 4])
            eng.tensor_add(t1, t1, t2)
            eng.tensor_add(s0, bno[:, sl, 2], bno[:, sl, 5])
            eng.tensor_scalar_mul(s0, s0, 0.125)
            eng.tensor_scalar_mul(t1, t1, 32.0)
            eng.tensor_add(ssf, t1, s0)
            a = sqp.tile([128, w], F32, name=f"a_{tag}")
            newton(eng, rinv2[:, sl], ssf, a)

        # ---- own rows (tiles 0..3): the whole chain runs IN ORDER on the
        # scalar engine (Square+accum -> Ln -> Exp(-1/2) -> scaled Copy),
        # immune to cross-engine scheduling; the act-table rsqrt (~1e-3)
        # only feeds the fp8 zoT, and a DVE Newton polish below refines
        # rinv2[0:4] for the exp scales. ----
        ss_own = persist.tile([128, 4], F32, name="ss_own")
        for t in range(4):
            sqo = sqp.tile([128, D], F32, name="sqo")
            nc.scalar.activation(
                sqo, yrall[:, t, :], AF.Square, accum_out=ss_own[:, t : t + 1]
            )
        lno = sqp.tile([128, 4], F32, name="lno")
        nc.scalar.activation(lno, ss_own, AF.Ln)
        rtab = persist.tile([128, 4], F32, name="rtab")
        nc.scalar.activation(rtab, lno, AF.Exp, scale=-0.5)
        zts = []
        for t in range(4):
            zt = zpool.tile([128, D], BF16, name=f"zt_{t}")
            if t < 2:
                nc.scalar.activation(
                    zt, yrall[:, t, :], AF.Copy, scale=rtab[:, t : t + 1]
                )
            else:
                nc.vector.tensor_scalar_mul(zt, yrall[:, t, :], rtab[:, t : t + 1])
            zts.append(zt)
        bn_tiles(range(4, 8))
        ssf_b = persist.tile([128, 4], F32, name="ssf_b")
        tb = sqp.tile([128, 4], F32, name="tb_b")
        tb2 = sqp.tile([128, 4], F32, name="tb2_b")
        nc.gpsimd.tensor_mul(tb, bno[:, 4:8, 1], bno[:, 4:8, 1])
        nc.gpsimd.tensor_mul(tb2, bno[:, 4:8, 4], bno[:, 4:8, 4])
        nc.gpsimd.tensor_add(tb, tb, tb2)
        nc.gpsimd.tensor_add(ssf_b, bno[:, 4:8, 2], bno[:, 4:8, 5])
        nc.gpsimd.tensor_scalar_mul(ssf_b, ssf_b, 0.125)
        nc.gpsimd.tensor_scalar_mul(tb, tb, 32.0)
        nc.gpsimd.tensor_add(ssf_b, ssf_b, tb)

        # rinv2[4:8] via the act tables: exp(-0.5 ln(ss/8) - ln2/2)
        # = 2/sqrt(ss); the two table swaps hide in the scalar idle window
        # between the zts copies and the transposes
        lnb4 = sqp.tile([128, 4], F32, name="lnb4")
        nc.scalar.activation(lnb4, ssf_b, AF.Ln)
        nc.scalar.activation(
            rinv2[:, 4:8], lnb4, AF.Exp, scale=-0.5, bias=bln2[:, 0:1]
        )

        # rinv2[0:4] = 2*rinv refined by one DVE Newton step off the
        # critical path (feeds only the exp scales, needed ~10us later)
        ssfa = sqp.tile([128, 4], F32, name="ssfa")
        nc.vector.tensor_scalar_mul(ssfa, ss_own, 0.125)
        r2A = rinv2[:, 0:4]
        aA = sqp.tile([128, 4], F32, name="aA")
        nc.vector.tensor_scalar_mul(r2A, rtab, 2.0)
        nc.vector.tensor_mul(aA, r2A, r2A)
        nc.vector.tensor_mul(aA, aA, ssfa)
        nc.vector.tensor_scalar(
            out=aA, in0=aA, scalar1=-1.0, scalar2=1.5,
            op0=ALU.mult, op1=ALU.add,
        )
        nc.vector.tensor_mul(r2A, r2A, aA)
        zoT = [persist.tile([128, 2, RPC], FP8, name=f"zoT_{m}") for m in range(2)]
        for k in range(4):
            ptr = ps_tr.tile([128, 4, 128], BF16, tag="tr", name="ptr")
            for t in range(4):
                nc.tensor.transpose(
                    ptr[:, t, :], zts[t][:, k * 128 : (k + 1) * 128], identw
                )
            if k < 2:
                nc.scalar.copy(zoT[k // 2][:, k % 2, :], ptr)
            else:
                nc.vector.tensor_copy(zoT[k // 2][:, k % 2, :], ptr)

        # ---- bag: normalize + transpose + S1T/Bgram ----
        bno_b = persist.tile([B, 6], F32, name="bno_b")
        nc.vector.bn_stats(bno_b, bag_t)
        tb1 = sqp.tile([B, 1], F32, name="tb1")
        tb2 = sqp.tile([B, 1], F32, name="tb2")
        ssfb = sqp.tile([B, 1], F32, name="ssfb")
        nc.gpsimd.tensor_mul(tb1, bno_b[:, 1:2], bno_b[:, 1:2])
        nc.gpsimd.tensor_mul(tb2, bno_b[:, 4:5], bno_b[:, 4:5])
        nc.gpsimd.tensor_add(tb1, tb1, tb2)
        nc.gpsimd.tensor_add(ssfb, bno_b[:, 2:3], bno_b[:, 5:6])
        nc.gpsimd.tensor_scalar_mul(ssfb, ssfb, 0.125)
        nc.gpsimd.tensor_scalar_mul(tb1, tb1, 32.0)
        nc.gpsimd.tensor_add(ssfb, ssfb, tb1)
        r2b = persist.tile([B, 1], F32, name="r2b")
        ab = sqp.tile([B, 1], F32, name="ab")
        newton(nc.gpsimd, r2b, ssfb, ab)
        zbag = persist.tile([B, D], BF16, name="zbag")
        nc.vector.tensor_scalar(
            out=zbag, in0=bag_t, scalar1=r2b[:, 0:1], scalar2=0.5,
            op0=ALU.mult, op1=ALU.mult,
        )
        zbagT = persist.tile([128, 4, B], BF16, name="zbagT")
        for k in range(4):
            pb = ps_sm.tile([128, B], BF16, tag="sm", name="pb")
            nc.tensor.transpose(pb, zbag[:, k * 128 : (k + 1) * 128], identb)
            nc.vector.tensor_copy(zbagT[:, k, :], pb)

        ps_s1 = ps_sm.tile([B, RPC], F32, tag="sm", name="ps_s1")
        for k in range(4):
            nc.tensor.matmul(
                ps_s1, lhsT=zbagT[:, k, :], rhs=zoT[k // 2][:, k % 2, :],
                start=(k == 0), stop=(k == 3),
            )
        s1sb = persist.tile([B, RPC], F32, name="s1sb")
        nc.vector.tensor_copy(s1sb, ps_s1)
        nc.gpsimd.dma_start(out=s1_d[:, :], in_=s1sb)

        ps_bg = ps_sm.tile([B, B], F32, tag="sm", name="ps_bg")
        for k in range(4):
            nc.tensor.matmul(
                ps_bg, lhsT=zbagT[:, k, :], rhs=zbagT[:, k, :],
                start=(k == 0), stop=(k == 3),
            )
        bgsb = persist.tile([B, B], F32, name="bgsb")
        nc.vector.tensor_copy(bgsb, ps_bg)
        nc.gpsimd.dma_start(out=bg_d[:, :], in_=bgsb)

        # ---- remaining sumsq + rinv batches (staggered) ----
        bn_tiles(range(8, 12))
        rsqrt_batch(slice(8, 12), "c")
        bn_tiles(range(12, 16))
        rsqrt_batch(slice(12, 16), "d")
        bn_tiles(range(16, 20))
        rsqrt_batch(slice(16, 20), "e")

        # ---- main loop: 20 column tiles ----
        # tiles 0..3 (diag block): colsum only, exp in place + accumulator
        # tiles 4..19: exp -> SBUF bf16 with accumulator (colsum) AND a
        #   ones-matmul row-partial into pr_main (d=1..3) or pr_4 (d=4)
        pd = persist.tile([128, CT], F32, name="pd")
        prsb = persist.tile([1, 2, RPC], F32, name="prsb")
        pr_main = ps_pr.tile([1, RPC], F32, name="pr_main")
        pr4 = ps_tr.tile([1, RPC], F32, tag="tr", name="pr4")
        TORDER = list(range(4, 16)) + [16, 17, 18, 19] + [0, 1, 2, 3]
        for t in TORDER:
            sl = slice(t * 128, (t + 1) * 128)
            pm = ps_g.tile([128, RPC], F32, name="pm")
            for m in range(2):
                nc.tensor.matmul(
                    pm, lhsT=ytm[m][:, :, sl], rhs=zoT[m],
                    start=(m == 0), stop=(m == 1), perf_mode=DR,
                )
            if t < 4:
                nc.scalar.activation(
                    pm, pm, AF.Exp, scale=rinv2[:, t : t + 1],
                    accum_out=pd[:, t : t + 1],
                )
                if t == 2:
                    nc.sync.dma_start(out=pd_d[:, 0:3], in_=pd[:, 0:3])
                if t == 3:
                    nc.sync.dma_start(out=pd_d[:, 3:4], in_=pd[:, 3:4])
            else:
                et = etp.tile([128, RPC], BF16, name="et")
                nc.scalar.activation(
                    et, pm, AF.Exp, scale=rinv2[:, t : t + 1],
                    accum_out=pd[:, t : t + 1],
                )
                if t < 16:
                    nc.tensor.matmul(
                        pr_main, lhsT=ones, rhs=et,
                        start=(t == 4), stop=(t == 15), skip_group_check=True,
                    )
                    if t == 15:
                        nc.vector.tensor_copy(prsb[:, 0, :], pr_main)
                        nc.sync.dma_start(
                            out=pr_d[:, 0:512], in_=prsb[:, 0, :]
                        )
                        nc.sync.dma_start(
                            out=pd_d[:, 4:16], in_=pd[:, 4:16]
                        )
                else:
                    nc.tensor.matmul(
                        pr4, lhsT=ones, rhs=et,
                        start=(t == 16), stop=(t == 19), skip_group_check=True,
                    )
                    if t == 19:
                        nc.vector.tensor_copy(prsb[:, 1, :], pr4)
                        nc.sync.dma_start(
                            out=pr_d[:, 512:1024], in_=prsb[:, 1, :]
                        )
                        nc.sync.dma_start(
                            out=pd_d[:, 16:CT], in_=pd[:, 16:CT]
                        )



def _get_nc():
    global _CACHED_NC
    if _CACHED_NC is None:
        nc = bacc.Bacc(
            "TRN2", target_bir_lowering=False, debug=False, num_devices=NCORES
        )
        nc = _build_kernel(nc)
        nc.compile()
        _CACHED_NC = nc
    return _CACHED_NC


def kernel(instance_emb: np.ndarray, bag_emb: np.ndarray) -> np.ndarray:
    global LAST_EXEC_TIME_NS
    Y = np.asarray(instance_emb, dtype=np.float32).reshape(BS, D)
    bg = np.ascontiguousarray(np.asarray(bag_emb, dtype=np.float32))

    in_maps = []
    for c in range(NCORES):
        Yc = np.roll(Y, -c * RPC, axis=0)
        # packed raw transpose (first 2560 rotated cols only):
        # ytm[p, j, col] = Yc[col, (2m+j)*128+p]
        T8 = np.ascontiguousarray(Yc[:CCOLS].T).astype(NP_FP8)  # [512, 2560]
        T8 = T8.reshape(2, 2, 128, CCOLS).transpose(0, 2, 1, 3)
        yrow = (
            Yc[:CCOLS].astype(NP_BF16).reshape(CT, 128, D).transpose(1, 0, 2)
        )
        in_maps.append(
            {
                "yt0": np.ascontiguousarray(T8[0]),
                "yt1": np.ascontiguousarray(T8[1]),
                "yrow": np.ascontiguousarray(yrow),
                "bag": bg,
            }
        )

    nc = _get_nc()
    trace = os.environ.get("CL_KERNEL_TRACE", "0") == "1"
    tmpdir = os.environ.get("CL_KERNEL_TRACE_DIR") or None
    if os.environ.get("CL_KERNEL_WARMUP", "0") == "1":
        bass_utils.run_bass_kernel_spmd(
            nc, in_maps, core_ids=list(range(NCORES)), trace=False
        )
    res = bass_utils.run_bass_kernel_spmd(
        nc, in_maps, core_ids=list(range(NCORES)), trace=trace, tmpdir=tmpdir
    )
    LAST_EXEC_TIME_NS = res.exec_time_ns

    return _assemble(res.results)


def _assemble(results) -> np.ndarray:
    """Host gather: sum the symmetric partial denominators, add the
    S1/Bgram terms, final log/sum in float64.

    Core X's pd[p, t] = sum over its 512 rows of E[c, r] for rotated
    column c = 128t + p (global (512X + 128t + p) mod 4096), covering
    relative block offsets d = t//4 in {0..4}.  pr[0] = row partials
    from offsets 1..3, pr[1] = from offset 4.  Offset-4 blocks are
    computed by both pair members; use the copies from cores 0..3.
    """
    denomG = np.zeros(BS, np.float64)
    S1 = np.zeros((BS, B), np.float64)
    pos = np.zeros(BS, np.float64)
    for c in range(NCORES):
        pdc = np.asarray(results[c]["pd"], np.float64)      # [128, 20]
        flat = pdc.T.reshape(CCOLS)
        if c >= 4:
            flat = flat.copy()
            flat[16 * 128 :] = 0.0                          # offset-4 dup
        full = np.zeros(BS, np.float64)
        full[:CCOLS] = flat
        denomG += np.roll(full, c * RPC)
        prc = np.asarray(results[c]["pr"], np.float64).reshape(2, RPC)
        own = prc[0] + (prc[1] if c < 4 else 0.0)
        denomG[c * RPC : (c + 1) * RPC] += own
        s1t = np.asarray(results[c]["s1t"], np.float64)     # [8, 512]
        S1[c * RPC : (c + 1) * RPC, :] = s1t.T
        pos[c * RPC : (c + 1) * RPC] = s1t[c, :]
    Bg = np.asarray(results[0]["bg"], np.float64)           # [8, 8]

    eS1 = np.exp(2.0 * S1)
    denom_i = denomG - E2 + 512.0 * np.sum(eS1, axis=1)
    denom_j = np.sum(eS1, axis=0) + 512.0 * np.sum(np.exp(2.0 * Bg), axis=1) - E2
    total = float(
        np.sum(np.log(denom_i)) - 4.0 * np.sum(pos) + 512.0 * np.sum(np.log(denom_j))
    )
    return np.float32(total / (2 * BS))


# revision 36
# speedup vs baseline: 1.0475x; 1.0475x over previous
"""Contrastive loss (NT-Xent style) Trainium2 kernel, 8-core SPMD.

Math: with z_i = normalize(instance_emb.reshape(4096, 512)),
zbag = normalize(bag_emb) [8, 512], z_j = repeat(zbag, 512) and
Z = [z_i; z_j] (8192 rows), the reference computes

  loss = (1/8192) * sum_r [ log(sum_{c != r} exp(2*sim[r,c])) - 2*pos[r] ]

with sim = Z @ Z.T, pos[r] = sim[r, r +- 4096].  Only the
G = z_i @ z_i.T quadrant (4096x4096) needs dense compute (the z_j half
has 8 distinct rows); S1 = z_i @ zbag.T [4096, 8] and Bg = zbag @ zbag.T
[8, 8] cover the rest:

  denom_i[r] = sum_c exp(2 G[r,c]) - e^2 + 512 * sum_b exp(2 S1[r,b])
  denom_j[b] = sum_r exp(2 S1[r,b]) + 512 * sum_b' exp(2 Bg[b,b']) - e^2
  loss*8192 = sum_r [log denom_i[r] - 4*S1[r, r//512]]
            + 512*sum_b log denom_j[b]

Distribution: NO collectives (the runtime entry barrier + AllGather
dominated the collective version).  Every core gets the full input,
rotated so its own 512 rows are local rows 0:511, and computes
E = exp(2 G) only for COLUMN blocks at relative offsets d in {0..4}
(20 column-tiles of 128).  E is symmetric, so each computed off-diagonal
entry serves two denominators: the activation accumulator gives the
column-block partials (denom of the E-row index) and a ones-matmul over
partitions gives the row partials (denom of the core's own rows).
Offsets 1..3 cover their mirror offsets 7..5; offset 4 is computed by
both members of each pair and the host discards the copy from cores
4..7.  The host sums the per-core partials (the gather/unshard step).

Per core c the host sends np.roll(Y, -512c):
  - yt0/yt1: RAW y^T fp8e4, packed for DoubleRow, only the first 2560
             rotated columns: ytm[p, j, col] = y[col, (2m+j)*128 + p]
  - yrow:    y row-major bf16 tiled [128, 20, 512] (tile t = rotated
             rows 128t..128t+127) -- feeds sumsq (rinv) and own z.
Device, per column-tile t (20 tiles):
  u[c, r] = sum_d y[c, d] * z_own[r, d]   (raw lhsT; 2 fp8 DR matmuls)
  E = exp(u * (2*rinv_c))                 (column norm folded into the
                                           ACT per-partition scale)
  pd[c, t] = sum_r E                      (ACT accumulator)
  tiles 4..19 also: E -> SBUF bf16, pr += ones^T @ E  (row partials,
  separate PSUM accumulators for d in {1,2,3} and d = 4)
Host: denom rows = rolled pd partials + own-row pr partials + S1/Bg
terms, then log/sum in float64.
"""

import os
import numpy as np
import ml_dtypes
from contextlib import ExitStack

import concourse.bass as bass
import concourse.bacc as bacc
import concourse.tile as tile
from concourse import mybir
from concourse import bass_utils
from concourse.masks import make_identity

F32 = mybir.dt.float32
BF16 = mybir.dt.bfloat16
FP8 = mybir.dt.float8e4

NP_FP8 = ml_dtypes.float8_e4m3
NP_BF16 = ml_dtypes.bfloat16

B, N, D = 8, 512, 512
BS = B * N              # 4096 instance rows
NCORES = 8
RPC = BS // NCORES      # 512 own rows per core
CT = 20                 # computed column tiles (offsets 0..4)
CCOLS = CT * 128        # 2560 columns
E2 = float(np.exp(2.0))
AF = mybir.ActivationFunctionType
ALU = mybir.AluOpType
DR = mybir.MatmulPerfMode.DoubleRow
SEED = 2.0 / float(np.sqrt(512.0))

LAST_EXEC_TIME_NS = None
_CACHED_NC = None


def _build_kernel(nc):
    yt0 = nc.dram_tensor("yt0", [128, 2, CCOLS], FP8, kind="ExternalInput")
    yt1 = nc.dram_tensor("yt1", [128, 2, CCOLS], FP8, kind="ExternalInput")
    yrow = nc.dram_tensor("yrow", [128, CT, D], BF16, kind="ExternalInput")
    bag = nc.dram_tensor("bag", [B, D], F32, kind="ExternalInput")
    pd_d = nc.dram_tensor("pd", [128, CT], F32, kind="ExternalOutput")
    pr_d = nc.dram_tensor("pr", [1, 2 * RPC], F32, kind="ExternalOutput")
    s1_d = nc.dram_tensor("s1t", [B, RPC], F32, kind="ExternalOutput")
    bg_d = nc.dram_tensor("bg", [B, B], F32, kind="ExternalOutput")

    with tile.TileContext(nc) as tc:
        _body(tc, yt0.ap(), yt1.ap(), yrow.ap(), bag.ap(),
              pd_d.ap(), pr_d.ap(), s1_d.ap(), bg_d.ap())
    return nc


def _body(tc, yt0, yt1, yrow, bag, pd_d, pr_d, s1_d, bg_d):
    nc = tc.nc
    with ExitStack() as ctx:
        consts = ctx.enter_context(tc.tile_pool(name="consts", bufs=1))
        persist = ctx.enter_context(tc.tile_pool(name="persist", bufs=1))
        zpool = ctx.enter_context(tc.tile_pool(name="zpool", bufs=4))
        sqp = ctx.enter_context(tc.tile_pool(name="sqp", bufs=2))
        etp = ctx.enter_context(tc.tile_pool(name="etp", bufs=4))
        ps_g = ctx.enter_context(tc.tile_pool(name="ps_g", bufs=4, space="PSUM"))
        ps_tr = ctx.enter_context(tc.tile_pool(name="ps_tr", bufs=2, space="PSUM"))
        ps_sm = ctx.enter_context(tc.tile_pool(name="ps_sm", bufs=1, space="PSUM"))
        ps_pr = ctx.enter_context(tc.tile_pool(name="ps_pr", bufs=1, space="PSUM"))

        identw = consts.tile([128, 128], BF16, name="identw")
        make_identity(nc, identw)
        identb = consts.tile([B, B], BF16, name="identb")
        make_identity(nc, identb)
        ones = consts.tile([128, 1], BF16, name="ones")
        nc.gpsimd.memset(ones, 1.0)
        # dummy Ln: hoists the natural_log act-table load (Square/Copy are
        # in that set too) into the idle DMA-wait window, leaving a single
        # on-chain table swap (to the exp set) later
        lnw = consts.tile([1, 1], F32, name="lnw")
        nc.scalar.activation(lnw, ones[0:1, 0:1], AF.Ln)

        # ---- input DMAs: own 4-tile slab first (it gates the prologue) ----
        yrall = persist.tile([128, CT, D], BF16, name="yrall")
        ytm = [persist.tile([128, 2, CCOLS], FP8, name=f"ytm_{m}") for m in range(2)]
        bag_t = persist.tile([B, D], F32, name="bag_t")

        nc.sync.dma_start(out=yrall[:, 0:4, :], in_=yrow[:, 0:4, :])
        yts = [yt0, yt1]
        for cchunk in range(2):
            sl = slice(cchunk * 1280, (cchunk + 1) * 1280)
            for m in range(2):
                nc.sync.dma_start(out=ytm[m][:, :, sl], in_=yts[m][:, :, sl])
        nc.scalar.dma_start(out=yrall[:, 4:12, :], in_=yrow[:, 4:12, :])
        nc.gpsimd.dma_start(out=bag_t, in_=bag[:, :])
        nc.gpsimd.dma_start(out=yrall[:, 12:20, :], in_=yrow[:, 12:20, :])

        bno = persist.tile([128, CT, 6], F32, name="bno")
        rinv2 = persist.tile([128, CT], F32, name="rinv2")  # 2/||row||

        def bn_tiles(ts):
            for t in ts:
                nc.vector.bn_stats(bno[:, t, :], yrall[:, t, :])

        # rinv2 = 2*ss^-1/2, table-free: sumsq of 512 N(0,1) terms is
        # 512 +- ~16%, so 3 Newton steps from the constant seed 2/sqrt(512)
        # converge to ~1e-5 (no scalar Ln/Exp => no act-table swaps).
        # In r2-space (r2 = 2r): r2 <- r2*(1.5 - (ss/8)*r2^2).
        def newton(eng, r2, ssf, a, iters=2):
            eng.tensor_scalar(
                out=r2, in0=ssf, scalar1=-(SEED ** 3), scalar2=1.5 * SEED,
                op0=ALU.mult, op1=ALU.add,
            )
            for _ in range(iters):
                eng.tensor_mul(a, r2, r2)
                eng.tensor_mul(a, a, ssf)
                eng.tensor_scalar(
                    out=a, in0=a, scalar1=-1.0, scalar2=1.5,
                    op0=ALU.mult, op1=ALU.add,
                )
                eng.tensor_mul(r2, r2, a)

        def rsqrt_batch(sl, tag, eng=None):
            # bn post: ss/8 = 32*(mu_e^2+mu_o^2) + (m2e+m2o)/8
            eng = eng or nc.gpsimd
            w = sl.stop - sl.start
            t1 = sqp.tile([128, w], F32, name=f"t1_{tag}")
            t2 = sqp.tile([128, w], F32, name=f"t2_{tag}")
            s0 = sqp.tile([128, w], F32, name=f"s0_{tag}")
            ssf = sqp.tile([128, w], F32, name=f"ssf_{tag}")
            eng.tensor_mul(t1, bno[:, sl, 1], bno[:, sl, 1])
            eng.tensor_mul(t2, bno[:, sl, 4], bno[:, sl, 4])
            eng.tensor_add(t1, t1, t2)
            eng.tensor_add(s0, bno[:, sl, 2], bno[:, sl, 5])
            eng.tensor_scalar_mul(s0, s0, 0.125)
            eng.tensor_scalar_mul(t1, t1, 32.0)
            eng.tensor_add(ssf, t1, s0)
            a = sqp.tile([128, w], F32, name=f"a_{tag}")
            newton(eng, rinv2[:, sl], ssf, a)

        # ---- own rows (tiles 0..3): the whole chain runs IN ORDER on the
        # scalar engine (Square+accum -> Ln -> Exp(-1/2) -> scaled Copy),
        # immune to cross-engine scheduling; the act-table rsqrt (~1e-3)
        # only feeds the fp8 zoT, and a DVE Newton polish below refines
        # rinv2[0:4] for the exp scales. ----
        ss_own = persist.tile([128, 4], F32, name="ss_own")
        for t in range(4):
            sqo = sqp.tile([128, D], F32, name="sqo")
            nc.scalar.activation(
                sqo, yrall[:, t, :], AF.Square, accum_out=ss_own[:, t : t + 1]
            )
        lno = sqp.tile([128, 4], F32, name="lno")
        nc.scalar.activation(lno, ss_own, AF.Ln)
        rtab = persist.tile([128, 4], F32, name="rtab")
        nc.scalar.activation(rtab, lno, AF.Exp, scale=-0.5)
        zts = []
        for t in range(4):
            zt = zpool.tile([128, D], BF16, name=f"zt_{t}")
            if t < 2:
                nc.scalar.activation(
                    zt, yrall[:, t, :], AF.Copy, scale=rtab[:, t : t + 1]
                )
            else:
                nc.vector.tensor_scalar_mul(zt, yrall[:, t, :], rtab[:, t : t + 1])
            zts.append(zt)
        # rinv2[0:4] = 2*rinv refined by one DVE Newton step off the
        # critical path (feeds only the exp scales, needed ~10us later)
        ssfa = sqp.tile([128, 4], F32, name="ssfa")
        nc.vector.tensor_scalar_mul(ssfa, ss_own, 0.125)
        r2A = rinv2[:, 0:4]
        aA = sqp.tile([128, 4], F32, name="aA")
        nc.vector.tensor_scalar_mul(r2A, rtab, 2.0)
        nc.vector.tensor_mul(aA, r2A, r2A)
        nc.vector.tensor_mul(aA, aA, ssfa)
        nc.vector.tensor_scalar(
            out=aA, in0=aA, scalar1=-1.0, scalar2=1.5,
            op0=ALU.mult, op1=ALU.add,
        )
        nc.vector.tensor_mul(r2A, r2A, aA)
        zoT = [persist.tile([128, 2, RPC], FP8, name=f"zoT_{m}") for m in range(2)]
        for k in range(4):
            ptr = ps_tr.tile([128, 4, 128], BF16, tag="tr", name="ptr")
            for t in range(4):
                nc.tensor.transpose(
                    ptr[:, t, :], zts[t][:, k * 128 : (k + 1) * 128], identw
                )
            if k < 2:
                nc.scalar.copy(zoT[k // 2][:, k % 2, :], ptr)
            else:
                nc.vector.tensor_copy(zoT[k // 2][:, k % 2, :], ptr)

        # ---- bag: normalize + transpose + S1T/Bgram ----
        bno_b = persist.tile([B, 6], F32, name="bno_b")
        nc.vector.bn_stats(bno_b, bag_t)
        tb1 = sqp.tile([B, 1], F32, name="tb1")
        tb2 = sqp.tile([B, 1], F32, name="tb2")
        ssfb = sqp.tile([B, 1], F32, name="ssfb")
        nc.gpsimd.tensor_mul(tb1, bno_b[:, 1:2], bno_b[:, 1:2])
        nc.gpsimd.tensor_mul(tb2, bno_b[:, 4:5], bno_b[:, 4:5])
        nc.gpsimd.tensor_add(tb1, tb1, tb2)
        nc.gpsimd.tensor_add(ssfb, bno_b[:, 2:3], bno_b[:, 5:6])
        nc.gpsimd.tensor_scalar_mul(ssfb, ssfb, 0.125)
        nc.gpsimd.tensor_scalar_mul(tb1, tb1, 32.0)
        nc.gpsimd.tensor_add(ssfb, ssfb, tb1)
        r2b = persist.tile([B, 1], F32, name="r2b")
        ab = sqp.tile([B, 1], F32, name="ab")
        newton(nc.gpsimd, r2b, ssfb, ab)
        zbag = persist.tile([B, D], BF16, name="zbag")
        nc.vector.tensor_scalar(
            out=zbag, in0=bag_t, scalar1=r2b[:, 0:1], scalar2=0.5,
            op0=ALU.mult, op1=ALU.mult,
        )
        zbagT = persist.tile([128, 4, B], BF16, name="zbagT")
        for k in range(4):
            pb = ps_sm.tile([128, B], BF16, tag="sm", name="pb")
            nc.tensor.transpose(pb, zbag[:, k * 128 : (k + 1) * 128], identb)
            nc.vector.tensor_copy(zbagT[:, k, :], pb)

        ps_s1 = ps_sm.tile([B, RPC], F32, tag="sm", name="ps_s1")
        for k in range(4):
            nc.tensor.matmul(
                ps_s1, lhsT=zbagT[:, k, :], rhs=zoT[k // 2][:, k % 2, :],
                start=(k == 0), stop=(k == 3),
            )
        s1sb = persist.tile([B, RPC], F32, name="s1sb")
        nc.vector.tensor_copy(s1sb, ps_s1)
        nc.gpsimd.dma_start(out=s1_d[:, :], in_=s1sb)

        ps_bg = ps_sm.tile([B, B], F32, tag="sm", name="ps_bg")
        for k in range(4):
            nc.tensor.matmul(
                ps_bg, lhsT=zbagT[:, k, :], rhs=zbagT[:, k, :],
                start=(k == 0), stop=(k == 3),
            )
        bgsb = persist.tile([B, B], F32, name="bgsb")
        nc.vector.tensor_copy(bgsb, ps_bg)
        nc.gpsimd.dma_start(out=bg_d[:, :], in_=bgsb)

        # ---- remaining sumsq + rinv batches (staggered) ----
        bn_tiles(range(4, 8))
        rsqrt_batch(slice(4, 8), "b")
        bn_tiles(range(8, 12))
        rsqrt_batch(slice(8, 12), "c")
        bn_tiles(range(12, 16))
        rsqrt_batch(slice(12, 16), "d")
        bn_tiles(range(16, 20))
        rsqrt_batch(slice(16, 20), "e")

        # ---- main loop: 20 column tiles ----
        # tiles 0..3 (diag block): colsum only, exp in place + accumulator
        # tiles 4..19: exp -> SBUF bf16 with accumulator (colsum) AND a
        #   ones-matmul row-partial into pr_main (d=1..3) or pr_4 (d=4)
        pd = persist.tile([128, CT], F32, name="pd")
        prsb = persist.tile([1, 2, RPC], F32, name="prsb")
        pr_main = ps_pr.tile([1, RPC], F32, name="pr_main")
        pr4 = ps_tr.tile([1, RPC], F32, tag="tr", name="pr4")
        TORDER = list(range(4, 16)) + [16, 17, 18, 19] + [0, 1, 2, 3]
        for t in TORDER:
            sl = slice(t * 128, (t + 1) * 128)
            pm = ps_g.tile([128, RPC], F32, name="pm")
            for m in range(2):
                nc.tensor.matmul(
                    pm, lhsT=ytm[m][:, :, sl], rhs=zoT[m],
                    start=(m == 0), stop=(m == 1), perf_mode=DR,
                )
            if t < 4:
                nc.scalar.activation(
                    pm, pm, AF.Exp, scale=rinv2[:, t : t + 1],
                    accum_out=pd[:, t : t + 1],
                )
                if t == 2:
                    nc.sync.dma_start(out=pd_d[:, 0:3], in_=pd[:, 0:3])
                if t == 3:
                    nc.sync.dma_start(out=pd_d[:, 3:4], in_=pd[:, 3:4])
            else:
                et = etp.tile([128, RPC], BF16, name="et")
                nc.scalar.activation(
                    et, pm, AF.Exp, scale=rinv2[:, t : t + 1],
                    accum_out=pd[:, t : t + 1],
                )
                if t < 16:
                    nc.tensor.matmul(
                        pr_main, lhsT=ones, rhs=et,
                        start=(t == 4), stop=(t == 15), skip_group_check=True,
                    )
                    if t == 15:
                        nc.vector.tensor_copy(prsb[:, 0, :], pr_main)
                        nc.sync.dma_start(
                            out=pr_d[:, 0:512], in_=prsb[:, 0, :]
                        )
                        nc.sync.dma_start(
                            out=pd_d[:, 4:16], in_=pd[:, 4:16]
                        )
                else:
                    nc.tensor.matmul(
                        pr4, lhsT=ones, rhs=et,
                        start=(t == 16), stop=(t == 19), skip_group_check=True,
                    )
                    if t == 19:
                        nc.vector.tensor_copy(prsb[:, 1, :], pr4)
                        nc.sync.dma_start(
                            out=pr_d[:, 512:1024], in_=prsb[:, 1, :]
                        )
                        nc.sync.dma_start(
                            out=pd_d[:, 16:CT], in_=pd[:, 16:CT]
                        )



def _get_nc():
    global _CACHED_NC
    if _CACHED_NC is None:
        nc = bacc.Bacc(
            "TRN2", target_bir_lowering=False, debug=False, num_devices=NCORES
        )
        nc = _build_kernel(nc)
        nc.compile()
        _CACHED_NC = nc
    return _CACHED_NC


def kernel(instance_emb: np.ndarray, bag_emb: np.ndarray) -> np.ndarray:
    global LAST_EXEC_TIME_NS
    Y = np.asarray(instance_emb, dtype=np.float32).reshape(BS, D)
    bg = np.ascontiguousarray(np.asarray(bag_emb, dtype=np.float32))

    in_maps = []
    for c in range(NCORES):
        Yc = np.roll(Y, -c * RPC, axis=0)
        # packed raw transpose (first 2560 rotated cols only):
        # ytm[p, j, col] = Yc[col, (2m+j)*128+p]
        T8 = np.ascontiguousarray(Yc[:CCOLS].T).astype(NP_FP8)  # [512, 2560]
        T8 = T8.reshape(2, 2, 128, CCOLS).transpose(0, 2, 1, 3)
        yrow = (
            Yc[:CCOLS].astype(NP_BF16).reshape(CT, 128, D).transpose(1, 0, 2)
        )
        in_maps.append(
            {
                "yt0": np.ascontiguousarray(T8[0]),
                "yt1": np.ascontiguousarray(T8[1]),
                "yrow": np.ascontiguousarray(yrow),
                "bag": bg,
            }
        )

    nc = _get_nc()
    trace = os.environ.get("CL_KERNEL_TRACE", "0") == "1"
    tmpdir = os.environ.get("CL_KERNEL_TRACE_DIR") or None
    if os.environ.get("CL_KERNEL_WARMUP", "0") == "1":
        bass_utils.run_bass_kernel_spmd(
            nc, in_maps, core_ids=list(range(NCORES)), trace=False
        )
    res = bass_utils.run_bass_kernel_spmd(
        nc, in_maps, core_ids=list(range(NCORES)), trace=trace, tmpdir=tmpdir
    )
    LAST_EXEC_TIME_NS = res.exec_time_ns

    return _assemble(res.results)


def _assemble(results) -> np.ndarray:
    """Host gather: sum the symmetric partial denominators, add the
    S1/Bgram terms, final log/sum in float64.

    Core X's pd[p, t] = sum over its 512 rows of E[c, r] for rotated
    column c = 128t + p (global (512X + 128t + p) mod 4096), covering
    relative block offsets d = t//4 in {0..4}.  pr[0] = row partials
    from offsets 1..3, pr[1] = from offset 4.  Offset-4 blocks are
    computed by both pair members; use the copies from cores 0..3.
    """
    denomG = np.zeros(BS, np.float64)
    S1 = np.zeros((BS, B), np.float64)
    pos = np.zeros(BS, np.float64)
    for c in range(NCORES):
        pdc = np.asarray(results[c]["pd"], np.float64)      # [128, 20]
        flat = pdc.T.reshape(CCOLS)
        if c >= 4:
            flat = flat.copy()
            flat[16 * 128 :] = 0.0                          # offset-4 dup
        full = np.zeros(BS, np.float64)
        full[:CCOLS] = flat
        denomG += np.roll(full, c * RPC)
        prc = np.asarray(results[c]["pr"], np.float64).reshape(2, RPC)
        own = prc[0] + (prc[1] if c < 4 else 0.0)
        denomG[c * RPC : (c + 1) * RPC] += own
        s1t = np.asarray(results[c]["s1t"], np.float64)     # [8, 512]
        S1[c * RPC : (c + 1) * RPC, :] = s1t.T
        pos[c * RPC : (c + 1) * RPC] = s1t[c, :]
    Bg = np.asarray(results[0]["bg"], np.float64)           # [8, 8]

    eS1 = np.exp(2.0 * S1)
    denom_i = denomG - E2 + 512.0 * np.sum(eS1, axis=1)
    denom_j = np.sum(eS1, axis=0) + 512.0 * np.sum(np.exp(2.0 * Bg), axis=1) - E2
    total = float(
        np.sum(np.log(denom_i)) - 4.0 * np.sum(pos) + 512.0 * np.sum(np.log(denom_j))
    )
    return np.float32(total / (2 * BS))


# revision 37
# speedup vs baseline: 1.1148x; 1.0643x over previous
"""Contrastive loss (NT-Xent style) Trainium2 kernel, 8-core SPMD.

Math: with z_i = normalize(instance_emb.reshape(4096, 512)),
zbag = normalize(bag_emb) [8, 512], z_j = repeat(zbag, 512) and
Z = [z_i; z_j] (8192 rows), the reference computes

  loss = (1/8192) * sum_r [ log(sum_{c != r} exp(2*sim[r,c])) - 2*pos[r] ]

with sim = Z @ Z.T, pos[r] = sim[r, r +- 4096].  Only the
G = z_i @ z_i.T quadrant (4096x4096) needs dense compute (the z_j half
has 8 distinct rows); S1 = z_i @ zbag.T [4096, 8] and Bg = zbag @ zbag.T
[8, 8] cover the rest:

  denom_i[r] = sum_c exp(2 G[r,c]) - e^2 + 512 * sum_b exp(2 S1[r,b])
  denom_j[b] = sum_r exp(2 S1[r,b]) + 512 * sum_b' exp(2 Bg[b,b']) - e^2
  loss*8192 = sum_r [log denom_i[r] - 4*S1[r, r//512]]
            + 512*sum_b log denom_j[b]

Distribution: NO collectives (the runtime entry barrier + AllGather
dominated the collective version).  Every core gets the full input,
rotated so its own 512 rows are local rows 0:511, and computes
E = exp(2 G) only for COLUMN blocks at relative offsets d in {0..4}
(20 column-tiles of 128).  E is symmetric, so each computed off-diagonal
entry serves two denominators: the activation accumulator gives the
column-block partials (denom of the E-row index) and a ones-matmul over
partitions gives the row partials (denom of the core's own rows).
Offsets 1..3 cover their mirror offsets 7..5; offset 4 is computed by
both members of each pair and the host discards the copy from cores
4..7.  The host sums the per-core partials (the gather/unshard step).

Per core c the host sends np.roll(Y, -512c):
  - yt0/yt1: RAW y^T fp8e4, packed for DoubleRow, only the first 2560
             rotated columns: ytm[p, j, col] = y[col, (2m+j)*128 + p]
  - yrow:    y row-major bf16 tiled [128, 20, 512] (tile t = rotated
             rows 128t..128t+127) -- feeds sumsq (rinv) and own z.
Device, per column-tile t (20 tiles):
  u[c, r] = sum_d y[c, d] * z_own[r, d]   (raw lhsT; 2 fp8 DR matmuls)
  E = exp(u * (2*rinv_c))                 (column norm folded into the
                                           ACT per-partition scale)
  pd[c, t] = sum_r E                      (ACT accumulator)
  tiles 4..19 also: E -> SBUF bf16, pr += ones^T @ E  (row partials,
  separate PSUM accumulators for d in {1,2,3} and d = 4)
Host: denom rows = rolled pd partials + own-row pr partials + S1/Bg
terms, then log/sum in float64.
"""

import os
import numpy as np
import ml_dtypes
from contextlib import ExitStack

import concourse.bass as bass
import concourse.bacc as bacc
import concourse.tile as tile
from concourse import mybir
from concourse import bass_utils
from concourse.masks import make_identity

F32 = mybir.dt.float32
BF16 = mybir.dt.bfloat16
FP8 = mybir.dt.float8e4

NP_FP8 = ml_dtypes.float8_e4m3
NP_BF16 = ml_dtypes.bfloat16

B, N, D = 8, 512, 512
BS = B * N              # 4096 instance rows
NCORES = 8
RPC = BS // NCORES      # 512 own rows per core
CT = 20                 # computed column tiles (offsets 0..4)
CCOLS = CT * 128        # 2560 columns
E2 = float(np.exp(2.0))
AF = mybir.ActivationFunctionType
ALU = mybir.AluOpType
DR = mybir.MatmulPerfMode.DoubleRow
SEED = 2.0 / float(np.sqrt(512.0))

LAST_EXEC_TIME_NS = None
_CACHED_NC = None


def _build_kernel(nc):
    yt0 = nc.dram_tensor("yt0", [128, 2, CCOLS], FP8, kind="ExternalInput")
    yt1 = nc.dram_tensor("yt1", [128, 2, CCOLS], FP8, kind="ExternalInput")
    yrow = nc.dram_tensor("yrow", [128, CT, D], BF16, kind="ExternalInput")
    bag = nc.dram_tensor("bag", [B, D], F32, kind="ExternalInput")
    pd_d = nc.dram_tensor("pd", [128, CT], F32, kind="ExternalOutput")
    pr_d = nc.dram_tensor("pr", [1, 2 * RPC], F32, kind="ExternalOutput")
    s1_d = nc.dram_tensor("s1t", [B, RPC], F32, kind="ExternalOutput")
    bg_d = nc.dram_tensor("bg", [B, B], F32, kind="ExternalOutput")

    with tile.TileContext(nc) as tc:
        _body(tc, yt0.ap(), yt1.ap(), yrow.ap(), bag.ap(),
              pd_d.ap(), pr_d.ap(), s1_d.ap(), bg_d.ap())
    return nc


def _body(tc, yt0, yt1, yrow, bag, pd_d, pr_d, s1_d, bg_d):
    nc = tc.nc
    with ExitStack() as ctx:
        consts = ctx.enter_context(tc.tile_pool(name="consts", bufs=1))
        persist = ctx.enter_context(tc.tile_pool(name="persist", bufs=1))
        zpool = ctx.enter_context(tc.tile_pool(name="zpool", bufs=4))
        sqp = ctx.enter_context(tc.tile_pool(name="sqp", bufs=2))
        etp = ctx.enter_context(tc.tile_pool(name="etp", bufs=4))
        ps_g = ctx.enter_context(tc.tile_pool(name="ps_g", bufs=4, space="PSUM"))
        ps_tr = ctx.enter_context(tc.tile_pool(name="ps_tr", bufs=2, space="PSUM"))
        ps_sm = ctx.enter_context(tc.tile_pool(name="ps_sm", bufs=1, space="PSUM"))
        ps_pr = ctx.enter_context(tc.tile_pool(name="ps_pr", bufs=1, space="PSUM"))

        identw = consts.tile([128, 128], BF16, name="identw")
        make_identity(nc, identw)
        identb = consts.tile([B, B], BF16, name="identb")
        make_identity(nc, identb)
        ones = consts.tile([128, 1], BF16, name="ones")
        nc.gpsimd.memset(ones, 1.0)
        # dummy Ln: hoists the natural_log act-table load (Square/Copy are
        # in that set too) into the idle DMA-wait window, leaving a single
        # on-chain table swap (to the exp set) later
        lnw = consts.tile([1, 1], F32, name="lnw")
        nc.scalar.activation(lnw, ones[0:1, 0:1], AF.Ln)

        # ---- input DMAs: own 4-tile slab first (it gates the prologue) ----
        yrall = persist.tile([128, CT, D], BF16, name="yrall")
        ytm = [persist.tile([128, 2, CCOLS], FP8, name=f"ytm_{m}") for m in range(2)]
        bag_t = persist.tile([B, D], F32, name="bag_t")

        nc.sync.dma_start(out=yrall[:, 0:4, :], in_=yrow[:, 0:4, :])
        yts = [yt0, yt1]
        for cchunk in range(2):
            sl = slice(cchunk * 1280, (cchunk + 1) * 1280)
            for m in range(2):
                nc.sync.dma_start(out=ytm[m][:, :, sl], in_=yts[m][:, :, sl])
        nc.scalar.dma_start(out=yrall[:, 4:12, :], in_=yrow[:, 4:12, :])
        nc.gpsimd.dma_start(out=bag_t, in_=bag[:, :])
        nc.gpsimd.dma_start(out=yrall[:, 12:20, :], in_=yrow[:, 12:20, :])

        bno = persist.tile([128, CT, 6], F32, name="bno")
        rinv2 = persist.tile([128, CT], F32, name="rinv2")  # 2/||row||

        def bn_tiles(ts):
            for t in ts:
                nc.vector.bn_stats(bno[:, t, :], yrall[:, t, :])

        # rinv2 = 2*ss^-1/2, table-free: sumsq of 512 N(0,1) terms is
        # 512 +- ~16%, so 3 Newton steps from the constant seed 2/sqrt(512)
        # converge to ~1e-5 (no scalar Ln/Exp => no act-table swaps).
        # In r2-space (r2 = 2r): r2 <- r2*(1.5 - (ss/8)*r2^2).
        def newton(eng, r2, ssf, a, iters=2):
            eng.tensor_scalar(
                out=r2, in0=ssf, scalar1=-(SEED ** 3), scalar2=1.5 * SEED,
                op0=ALU.mult, op1=ALU.add,
            )
            for _ in range(iters):
                eng.tensor_mul(a, r2, r2)
                eng.tensor_mul(a, a, ssf)
                eng.tensor_scalar(
                    out=a, in0=a, scalar1=-1.0, scalar2=1.5,
                    op0=ALU.mult, op1=ALU.add,
                )
                eng.tensor_mul(r2, r2, a)

        def rsqrt_batch(sl, tag, eng=None):
            # bn post: ss/8 = 32*(mu_e^2+mu_o^2) + (m2e+m2o)/8
            eng = eng or nc.gpsimd
            w = sl.stop - sl.start
            t1 = sqp.tile([128, w], F32, name=f"t1_{tag}")
            t2 = sqp.tile([128, w], F32, name=f"t2_{tag}")
            s0 = sqp.tile([128, w], F32, name=f"s0_{tag}")
            ssf = sqp.tile([128, w], F32, name=f"ssf_{tag}")
            eng.tensor_mul(t1, bno[:, sl, 1], bno[:, sl, 1])
            eng.tensor_mul(t2, bno[:, sl, 4], bno[:, sl, 4])
            eng.tensor_add(t1, t1, t2)
            eng.tensor_add(s0, bno[:, sl, 2], bno[:, sl, 5])
            eng.tensor_scalar_mul(s0, s0, 0.125)
            eng.tensor_scalar_mul(t1, t1, 32.0)
            eng.tensor_add(ssf, t1, s0)
            a = sqp.tile([128, w], F32, name=f"a_{tag}")
            newton(eng, rinv2[:, sl], ssf, a)

        # ---- own rows (tiles 0..3): the whole chain runs IN ORDER on the
        # scalar engine (Square+accum -> Ln -> Exp(-1/2) -> scaled Copy),
        # immune to cross-engine scheduling; the act-table rsqrt (~1e-3)
        # only feeds the fp8 zoT, and a DVE Newton polish below refines
        # rinv2[0:4] for the exp scales. ----
        ss_own = persist.tile([128, 4], F32, name="ss_own")
        for t in range(4):
            sqo = sqp.tile([128, D], F32, name="sqo")
            nc.scalar.activation(
                sqo, yrall[:, t, :], AF.Square, accum_out=ss_own[:, t : t + 1]
            )
        lno = sqp.tile([128, 4], F32, name="lno")
        nc.scalar.activation(lno, ss_own, AF.Ln)
        rtab = persist.tile([128, 4], F32, name="rtab")
        nc.scalar.activation(rtab, lno, AF.Exp, scale=-0.5)
        zts = []
        for t in range(4):
            zt = zpool.tile([128, D], BF16, name=f"zt_{t}")
            if t < 2:
                nc.scalar.activation(
                    zt, yrall[:, t, :], AF.Copy, scale=rtab[:, t : t + 1]
                )
            else:
                nc.vector.tensor_scalar_mul(zt, yrall[:, t, :], rtab[:, t : t + 1])
            zts.append(zt)
        # rinv2[0:4] = 2*rinv refined by one DVE Newton step off the
        # critical path (feeds only the exp scales, needed ~10us later)
        ssfa = sqp.tile([128, 4], F32, name="ssfa")
        nc.vector.tensor_scalar_mul(ssfa, ss_own, 0.125)
        r2A = rinv2[:, 0:4]
        aA = sqp.tile([128, 4], F32, name="aA")
        nc.vector.tensor_scalar_mul(r2A, rtab, 2.0)
        nc.vector.tensor_mul(aA, r2A, r2A)
        nc.vector.tensor_mul(aA, aA, ssfa)
        nc.vector.tensor_scalar(
            out=aA, in0=aA, scalar1=-1.0, scalar2=1.5,
            op0=ALU.mult, op1=ALU.add,
        )
        nc.vector.tensor_mul(r2A, r2A, aA)
        zoT = [persist.tile([128, 2, RPC], FP8, name=f"zoT_{m}") for m in range(2)]
        for k in range(4):
            ptr = ps_tr.tile([128, 4, 128], BF16, tag="tr", name="ptr")
            for t in range(4):
                nc.tensor.transpose(
                    ptr[:, t, :], zts[t][:, k * 128 : (k + 1) * 128], identw
                )
            if k < 2:
                nc.scalar.copy(zoT[k // 2][:, k % 2, :], ptr)
            else:
                nc.vector.tensor_copy(zoT[k // 2][:, k % 2, :], ptr)

        # ---- remaining sumsq + rinv batches (staggered) ----
        bn_tiles(range(4, 8))
        rsqrt_batch(slice(4, 8), "b")
        bn_tiles(range(8, 12))
        rsqrt_batch(slice(8, 12), "c")
        bn_tiles(range(12, 16))
        rsqrt_batch(slice(12, 16), "d")
        bn_tiles(range(16, 20))
        rsqrt_batch(slice(16, 20), "e")

        # ---- main loop: 20 column tiles ----
        # tiles 0..3 (diag block): colsum only, exp in place + accumulator
        # tiles 4..19: exp -> SBUF bf16 with accumulator (colsum) AND a
        #   ones-matmul row-partial into pr_main (d=1..3) or pr_4 (d=4)
        pd = persist.tile([128, CT], F32, name="pd")
        prsb = persist.tile([1, 2, RPC], F32, name="prsb")
        pr_main = ps_pr.tile([1, RPC], F32, name="pr_main")
        pr4 = ps_tr.tile([1, RPC], F32, tag="tr", name="pr4")
        TORDER = list(range(4, 16)) + [16, 17, 18, 19] + [0, 1, 2, 3]
        for t in TORDER:
            sl = slice(t * 128, (t + 1) * 128)
            pm = ps_g.tile([128, RPC], F32, name="pm")
            for m in range(2):
                nc.tensor.matmul(
                    pm, lhsT=ytm[m][:, :, sl], rhs=zoT[m],
                    start=(m == 0), stop=(m == 1), perf_mode=DR,
                )
            if t < 4:
                nc.scalar.activation(
                    pm, pm, AF.Exp, scale=rinv2[:, t : t + 1],
                    accum_out=pd[:, t : t + 1],
                )
                if t == 2:
                    nc.sync.dma_start(out=pd_d[:, 0:3], in_=pd[:, 0:3])
                if t == 3:
                    nc.sync.dma_start(out=pd_d[:, 3:4], in_=pd[:, 3:4])
            else:
                et = etp.tile([128, RPC], BF16, name="et")
                nc.scalar.activation(
                    et, pm, AF.Exp, scale=rinv2[:, t : t + 1],
                    accum_out=pd[:, t : t + 1],
                )
                if t < 16:
                    nc.tensor.matmul(
                        pr_main, lhsT=ones, rhs=et,
                        start=(t == 4), stop=(t == 15), skip_group_check=True,
                    )
                    if t == 15:
                        nc.vector.tensor_copy(prsb[:, 0, :], pr_main)
                        nc.sync.dma_start(
                            out=pr_d[:, 0:512], in_=prsb[:, 0, :]
                        )
                        nc.sync.dma_start(
                            out=pd_d[:, 4:16], in_=pd[:, 4:16]
                        )
                else:
                    nc.tensor.matmul(
                        pr4, lhsT=ones, rhs=et,
                        start=(t == 16), stop=(t == 19), skip_group_check=True,
                    )
                    if t == 19:
                        nc.vector.tensor_copy(prsb[:, 1, :], pr4)
                        nc.sync.dma_start(
                            out=pr_d[:, 512:1024], in_=prsb[:, 1, :]
                        )
                        nc.sync.dma_start(
                            out=pd_d[:, 16:CT], in_=pd[:, 16:CT]
                        )
        # ---- bag: normalize + transpose + S1T/Bgram ----
        bno_b = persist.tile([B, 6], F32, name="bno_b")
        nc.vector.bn_stats(bno_b, bag_t)
        tb1 = sqp.tile([B, 1], F32, name="tb1")
        tb2 = sqp.tile([B, 1], F32, name="tb2")
        ssfb = sqp.tile([B, 1], F32, name="ssfb")
        nc.gpsimd.tensor_mul(tb1, bno_b[:, 1:2], bno_b[:, 1:2])
        nc.gpsimd.tensor_mul(tb2, bno_b[:, 4:5], bno_b[:, 4:5])
        nc.gpsimd.tensor_add(tb1, tb1, tb2)
        nc.gpsimd.tensor_add(ssfb, bno_b[:, 2:3], bno_b[:, 5:6])
        nc.gpsimd.tensor_scalar_mul(ssfb, ssfb, 0.125)
        nc.gpsimd.tensor_scalar_mul(tb1, tb1, 32.0)
        nc.gpsimd.tensor_add(ssfb, ssfb, tb1)
        r2b = persist.tile([B, 1], F32, name="r2b")
        ab = sqp.tile([B, 1], F32, name="ab")
        newton(nc.gpsimd, r2b, ssfb, ab)
        zbag = persist.tile([B, D], BF16, name="zbag")
        nc.vector.tensor_scalar(
            out=zbag, in0=bag_t, scalar1=r2b[:, 0:1], scalar2=0.5,
            op0=ALU.mult, op1=ALU.mult,
        )
        zbagT = persist.tile([128, 4, B], BF16, name="zbagT")
        for k in range(4):
            pb = ps_sm.tile([128, B], BF16, tag="sm", name="pb")
            nc.tensor.transpose(pb, zbag[:, k * 128 : (k + 1) * 128], identb)
            nc.vector.tensor_copy(zbagT[:, k, :], pb)

        ps_s1 = ps_sm.tile([B, RPC], F32, tag="sm", name="ps_s1")
        for k in range(4):
            nc.tensor.matmul(
                ps_s1, lhsT=zbagT[:, k, :], rhs=zoT[k // 2][:, k % 2, :],
                start=(k == 0), stop=(k == 3),
            )
        s1sb = persist.tile([B, RPC], F32, name="s1sb")
        nc.vector.tensor_copy(s1sb, ps_s1)
        nc.gpsimd.dma_start(out=s1_d[:, :], in_=s1sb)

        ps_bg = ps_sm.tile([B, B], F32, tag="sm", name="ps_bg")
        for k in range(4):
            nc.tensor.matmul(
                ps_bg, lhsT=zbagT[:, k, :], rhs=zbagT[:, k, :],
                start=(k == 0), stop=(k == 3),
            )
        bgsb = persist.tile([B, B], F32, name="bgsb")
        nc.vector.tensor_copy(bgsb, ps_bg)
        nc.gpsimd.dma_start(out=bg_d[:, :], in_=bgsb)





def _get_nc():
    global _CACHED_NC
    if _CACHED_NC is None:
        nc = bacc.Bacc(
            "TRN2", target_bir_lowering=False, debug=False, num_devices=NCORES
        )
        nc = _build_kernel(nc)
        nc.compile()
        _CACHED_NC = nc
    return _CACHED_NC


def kernel(instance_emb: np.ndarray, bag_emb: np.ndarray) -> np.ndarray:
    global LAST_EXEC_TIME_NS
    Y = np.asarray(instance_emb, dtype=np.float32).reshape(BS, D)
    bg = np.ascontiguousarray(np.asarray(bag_emb, dtype=np.float32))

    in_maps = []
    for c in range(NCORES):
        Yc = np.roll(Y, -c * RPC, axis=0)
        # packed raw transpose (first 2560 rotated cols only):
        # ytm[p, j, col] = Yc[col, (2m+j)*128+p]
        T8 = np.ascontiguousarray(Yc[:CCOLS].T).astype(NP_FP8)  # [512, 2560]
        T8 = T8.reshape(2, 2, 128, CCOLS).transpose(0, 2, 1, 3)
        yrow = (
            Yc[:CCOLS].astype(NP_BF16).reshape(CT, 128, D).transpose(1, 0, 2)
        )
        in_maps.append(
            {
                "yt0": np.ascontiguousarray(T8[0]),
                "yt1": np.ascontiguousarray(T8[1]),
                "yrow": np.ascontiguousarray(yrow),
                "bag": bg,
            }
        )

    nc = _get_nc()
    trace = os.environ.get("CL_KERNEL_TRACE", "0") == "1"
    tmpdir = os.environ.get("CL_KERNEL_TRACE_DIR") or None
    if os.environ.get("CL_KERNEL_WARMUP", "0") == "1":
        bass_utils.run_bass_kernel_spmd(
            nc, in_maps, core_ids=list(range(NCORES)), trace=False
        )
    res = bass_utils.run_bass_kernel_spmd(
        nc, in_maps, core_ids=list(range(NCORES)), trace=trace, tmpdir=tmpdir
    )
    LAST_EXEC_TIME_NS = res.exec_time_ns

    return _assemble(res.results)


def _assemble(results) -> np.ndarray:
    """Host gather: sum the symmetric partial denominators, add the
    S1/Bgram terms, final log/sum in float64.

    Core X's pd[p, t] = sum over its 512 rows of E[c, r] for rotated
    column c = 128t + p (global (512X + 128t + p) mod 4096), covering
    relative block offsets d = t//4 in {0..4}.  pr[0] = row partials
    from offsets 1..3, pr[1] = from offset 4.  Offset-4 blocks are
    computed by both pair members; use the copies from cores 0..3.
    """
    denomG = np.zeros(BS, np.float64)
    S1 = np.zeros((BS, B), np.float64)
    pos = np.zeros(BS, np.float64)
    for c in range(NCORES):
        pdc = np.asarray(results[c]["pd"], np.float64)      # [128, 20]
        flat = pdc.T.reshape(CCOLS)
        if c >= 4:
            flat = flat.copy()
            flat[16 * 128 :] = 0.0                          # offset-4 dup
        full = np.zeros(BS, np.float64)
        full[:CCOLS] = flat
        denomG += np.roll(full, c * RPC)
        prc = np.asarray(results[c]["pr"], np.float64).reshape(2, RPC)
        own = prc[0] + (prc[1] if c < 4 else 0.0)
        denomG[c * RPC : (c + 1) * RPC] += own
        s1t = np.asarray(results[c]["s1t"], np.float64)     # [8, 512]
        S1[c * RPC : (c + 1) * RPC, :] = s1t.T
        pos[c * RPC : (c + 1) * RPC] = s1t[c, :]
    Bg = np.asarray(results[0]["bg"], np.float64)           # [8, 8]

    eS1 = np.exp(2.0 * S1)
    denom_i = denomG - E2 + 512.0 * np.sum(eS1, axis=1)
    denom_j = np.sum(eS1, axis=0) + 512.0 * np.sum(np.exp(2.0 * Bg), axis=1) - E2
    total = float(
        np.sum(np.log(denom_i)) - 4.0 * np.sum(pos) + 512.0 * np.sum(np.log(denom_j))
    )
    return np.float32(total / (2 * BS))
